# revision 1
# baseline (speedup 1.0000x reference)
"""Trainium2 Bass kernel for nn_Classifier (GNN edge-MLP link predictor).

Computes, for E candidate edges:
    out[e] = W2 . relu( x_nc[i0[e]] @ W1[:H] + x_pr[i1[e]] @ W1[H:] + b1 ) + b2

Strategy (8 NeuronCores, data-parallel over edges):
  - Edges are sharded across the 8 cores (125k edges each).
  - Node tables are replicated, stored bf16 in DRAM.
  - Per tile of T edges: gpsimd.dma_gather(transpose=True) pulls the bf16
    feature rows for each endpoint directly into feature-major layout
    [H=128 partitions, T edges] in SBUF, so fc1 runs straight on the
    tensor engine with W1 as the stationary operand (no on-chip transpose).
  - relu(+b1) on ScalarE/VectorE (alternating), cast to bf16.
  - fc2 is a K=128, M=1 matmul; the [1, chunk] PSUM rows are drained
    (+b2) to SBUF by VectorE/ScalarE and DMAed out.

All matmuls are bf16 with fp32 PSUM accumulation (measured end-to-end
error vs the fp32 reference: ~4e-3 of output scale).
"""

import math

import numpy as np
import ml_dtypes

import concourse.bass as bass
import concourse.tile as tile
from concourse import bacc, mybir
from concourse import bass_utils

F32 = mybir.dt.float32
BF16 = mybir.dt.bfloat16
I16 = mybir.dt.int16

N_CORES = 8
H = 128

# Full-problem geometry (hardcoded per the task contract).
E_TOTAL = 1_000_000
N_NODES = 20_000


def _build(n_nodes: int, e_pad: int, t_gather: int, chunk: int, reps: int = 1):
    """Build + compile the per-core SPMD program.

    n_nodes: rows in each node table
    e_pad:   padded per-core edge count (multiple of t_gather)
    t_gather: edges per dma_gather instruction (multiple of 128)
    chunk:   edges per matmul (<=512, divides t_gather)
    reps:    repeat the edge loop (timing-harness use only)
    """
    assert e_pad % t_gather == 0 and t_gather % 128 == 0
    assert chunk <= 512 and t_gather % chunk == 0
    n_tiles = e_pad // t_gather

    nc = bacc.Bacc(
        "TRN2",
        target_bir_lowering=False,
        debug=False,
        num_devices=N_CORES,
    )

    t_nc = nc.dram_tensor("t_nc", [n_nodes, H], BF16, kind="ExternalInput").ap()
    t_pr = nc.dram_tensor("t_pr", [n_nodes, H], BF16, kind="ExternalInput").ap()
    idx0 = nc.dram_tensor("idx0", [16, e_pad // 16], I16, kind="ExternalInput").ap()
    idx1 = nc.dram_tensor("idx1", [16, e_pad // 16], I16, kind="ExternalInput").ap()
    w1 = nc.dram_tensor("w1", [2 * H, H], BF16, kind="ExternalInput").ap()
    b1 = nc.dram_tensor("b1", [H, 1], F32, kind="ExternalInput").ap()
    w2 = nc.dram_tensor("w2", [H, 1], BF16, kind="ExternalInput").ap()
    b2 = nc.dram_tensor("b2", [1, 1], F32, kind="ExternalInput").ap()
    out = nc.dram_tensor("out", [1, e_pad], F32, kind="ExternalOutput").ap()

    relu = mybir.ActivationFunctionType.Relu
    ident = mybir.ActivationFunctionType.Identity
    add_op = mybir.AluOpType.add
    max_op = mybir.AluOpType.max

    with tile.TileContext(nc) as tc:
        with (
            tc.tile_pool(name="const", bufs=1) as cpool,
            tc.tile_pool(name="idx", bufs=1) as ipool,
            tc.tile_pool(name="gather", bufs=2) as gpool,
            tc.tile_pool(name="h", bufs=2) as hpool,
            tc.tile_pool(name="stage", bufs=2) as spool,
            tc.tile_pool(name="fc1ps", bufs=4, space="PSUM") as fc1pool,
            tc.tile_pool(name="fc2ps", bufs=3, space="PSUM") as fc2pool,
        ):
            # ---- constants ----
            w1nc = cpool.tile([H, H], BF16, tag="w1nc")
            nc.sync.dma_start(w1nc[:], w1[0:H, :])
            w1pr = cpool.tile([H, H], BF16, tag="w1pr")
            nc.sync.dma_start(w1pr[:], w1[H : 2 * H, :])
            b1_sb = cpool.tile([H, 1], F32, tag="b1")
            nc.sync.dma_start(b1_sb[:], b1[:])
            w2_sb = cpool.tile([H, 1], BF16, tag="w2")
            nc.sync.dma_start(w2_sb[:], w2[:])
            b2_sb = cpool.tile([1, 1], F32, tag="b2")
            nc.sync.dma_start(b2_sb[:], b2[:])

            # ---- indices: replicate [16, N] across the 8 partition groups ----
            idx0_sb = ipool.tile([128, e_pad // 16], I16, tag="idx0")
            idx1_sb = ipool.tile([128, e_pad // 16], I16, tag="idx1")
            for k in range(8):
                nc.sync.dma_start(idx0_sb[16 * k : 16 * (k + 1), :], idx0[:])
                nc.sync.dma_start(idx1_sb[16 * k : 16 * (k + 1), :], idx1[:])

            ic = t_gather // 16  # idx columns per gather tile

            for t in [t for _ in range(reps) for t in range(n_tiles)]:
                g_nc = gpool.tile([H, t_gather], BF16, tag="g_nc")
                nc.gpsimd.dma_gather(
                    g_nc[:].rearrange("p (one t) -> p one t", one=1),
                    t_nc,
                    idx0_sb[:, t * ic : (t + 1) * ic],
                    t_gather,
                    t_gather,
                    H,
                    transpose=True,
                    single_packet=(t_gather <= 512),
                )
                g_pr = gpool.tile([H, t_gather], BF16, tag="g_pr")
                nc.gpsimd.dma_gather(
                    g_pr[:].rearrange("p (one t) -> p one t", one=1),
                    t_pr,
                    idx1_sb[:, t * ic : (t + 1) * ic],
                    t_gather,
                    t_gather,
                    H,
                    transpose=True,
                    single_packet=(t_gather <= 512),
                )

                h_sb = hpool.tile([H, t_gather], BF16, tag="h")
                stage = spool.tile([1, t_gather], F32, tag="stage")

                for c in range(t_gather // chunk):
                    sl = slice(c * chunk, (c + 1) * chunk)
                    ps = fc1pool.tile([H, chunk], F32, tag="fc1")
                    nc.tensor.matmul(
                        ps[:], w1nc[:], g_nc[:, sl], start=True, stop=False
                    )
                    nc.tensor.matmul(
                        ps[:], w1pr[:], g_pr[:, sl], start=False, stop=True
                    )
                    # relu(ps + b1) -> bf16, alternating engines
                    if c % 2 == 0:
                        nc.scalar.activation(h_sb[:, sl], ps[:], relu, bias=b1_sb[:])
                    else:
                        nc.vector.tensor_scalar(
                            h_sb[:, sl], ps[:], b1_sb[:], 0.0, add_op, max_op
                        )

                    ps2 = fc2pool.tile([1, chunk], F32, tag="fc2")
                    nc.tensor.matmul(
                        ps2[:], w2_sb[:], h_sb[:, sl], start=True, stop=True
                    )
                    # stage = ps2 + b2, opposite-parity engines
                    if c % 2 == 0:
                        nc.vector.tensor_scalar(
                            stage[:, sl], ps2[:], b2_sb[:], None, add_op
                        )
                    else:
                        nc.scalar.activation(stage[:, sl], ps2[:], ident, bias=b2_sb[:])

                nc.sync.dma_start(out[:, t * t_gather : (t + 1) * t_gather], stage[:])

    nc.compile()
    return nc


# ---------------------------------------------------------------------------
# Host-side wrapper
# ---------------------------------------------------------------------------

_CACHE: dict = {}


def _wrap_idx(idx: np.ndarray, e_pad: int) -> np.ndarray:
    """int16 [16, e_pad//16] with index i at [i % 16, i // 16]."""
    pad = np.zeros(e_pad, np.int16)
    pad[: idx.shape[0]] = idx.astype(np.int16)
    return np.ascontiguousarray(pad.reshape(e_pad // 16, 16).T)


def _get_program(n_nodes, e_pad, t_gather, chunk):
    key = (n_nodes, e_pad, t_gather, chunk)
    if key not in _CACHE:
        _CACHE[key] = _build(n_nodes, e_pad, t_gather, chunk)
    return _CACHE[key]


def kernel(
    x_ncRNA: np.ndarray,
    x_Protein: np.ndarray,
    edge_label_index: np.ndarray,
    W1: np.ndarray,
    b1: np.ndarray,
    W2: np.ndarray,
    b2: np.ndarray,
    _t_gather: int = 8192,
    _chunk: int = 512,
    _trace: bool = False,
) -> np.ndarray:
    E = edge_label_index.shape[1]
    n_nodes = x_ncRNA.shape[0]
    assert E % N_CORES == 0
    e_core = E // N_CORES
    n_tiles = math.ceil(e_core / _t_gather)
    e_pad = n_tiles * _t_gather

    nc = _get_program(n_nodes, e_pad, _t_gather, _chunk)

    t_nc = np.ascontiguousarray(x_ncRNA.astype(ml_dtypes.bfloat16))
    t_pr = np.ascontiguousarray(x_Protein.astype(ml_dtypes.bfloat16))
    w1 = np.ascontiguousarray(W1.astype(ml_dtypes.bfloat16))
    w2 = np.ascontiguousarray(W2.astype(ml_dtypes.bfloat16))
    b1_ = np.ascontiguousarray(b1.reshape(H, 1).astype(np.float32))
    b2_ = np.ascontiguousarray(b2.reshape(1, 1).astype(np.float32))

    in_maps = []
    for c in range(N_CORES):
        sl = slice(c * e_core, (c + 1) * e_core)
        in_maps.append(
            {
                "t_nc": t_nc,
                "t_pr": t_pr,
                "idx0": _wrap_idx(np.asarray(edge_label_index[0, sl]), e_pad),
                "idx1": _wrap_idx(np.asarray(edge_label_index[1, sl]), e_pad),
                "w1": w1,
                "b1": b1_,
                "w2": w2,
                "b2": b2_,
            }
        )

    res = bass_utils.run_bass_kernel_spmd(
        nc, in_maps, core_ids=list(range(N_CORES)), trace=_trace
    )
    out = np.empty(E, np.float32)
    for c in range(N_CORES):
        out[c * e_core : (c + 1) * e_core] = res.results[c]["out"][0, :e_core]
    kernel._last_results = res
    return out



# revision 2
# speedup vs baseline: 5.3796x; 5.3796x over previous
"""Trainium2 Bass kernel for nn_Classifier (GNN edge-MLP link predictor).

Computes, for E candidate edges:
    out[e] = W2 . relu( x_nc[i0[e]] @ W1[:H] + x_pr[i1[e]] @ W1[H:] + b1 ) + b2

Strategy (8 NeuronCores, data-parallel over edges):
  - Edges are sharded across the 8 cores (125k edges each).
  - Node tables are replicated, stored bf16 in DRAM.
  - Per tile of T edges: gpsimd.dma_gather(transpose=True) pulls the bf16
    feature rows for each endpoint directly into feature-major layout
    [H=128 partitions, T edges] in SBUF, so fc1 runs straight on the
    tensor engine with W1 as the stationary operand (no on-chip transpose).
  - relu(+b1) on ScalarE/VectorE (alternating), cast to bf16.
  - fc2 is a K=128, M=1 matmul; the [1, chunk] PSUM rows are drained
    (+b2) to SBUF by VectorE/ScalarE and DMAed out.

All matmuls are bf16 with fp32 PSUM accumulation (measured end-to-end
error vs the fp32 reference: ~4e-3 of output scale).
"""

import math

import numpy as np
import ml_dtypes

import concourse.bass as bass
import concourse.tile as tile
from concourse import bacc, mybir
from concourse import bass_utils

F32 = mybir.dt.float32
BF16 = mybir.dt.bfloat16
I16 = mybir.dt.int16

N_CORES = 8
H = 128

# Full-problem geometry (hardcoded per the task contract).
E_TOTAL = 1_000_000
N_NODES = 20_000


def _build(n_nodes: int, e_pad: int, t_gather: int, chunk: int, reps: int = 1):
    """Build + compile the per-core SPMD program.

    n_nodes: rows in each node table
    e_pad:   padded per-core edge count (multiple of t_gather)
    t_gather: edges per dma_gather instruction (multiple of 128)
    chunk:   edges per matmul (<=512, divides t_gather)
    reps:    repeat the edge loop (timing-harness use only)
    """
    assert e_pad % t_gather == 0 and t_gather % 128 == 0
    assert chunk <= 512 and t_gather % chunk == 0
    n_tiles = e_pad // t_gather

    nc = bacc.Bacc(
        "TRN2",
        target_bir_lowering=False,
        debug=False,
        num_devices=N_CORES,
        num_swdge_queues=4,
    )

    t_nc = nc.dram_tensor("t_nc", [n_nodes, H], BF16, kind="ExternalInput").ap()
    t_pr = nc.dram_tensor("t_pr", [n_nodes, H], BF16, kind="ExternalInput").ap()
    idx0 = nc.dram_tensor("idx0", [16, e_pad // 16], I16, kind="ExternalInput").ap()
    idx1 = nc.dram_tensor("idx1", [16, e_pad // 16], I16, kind="ExternalInput").ap()
    w1 = nc.dram_tensor("w1", [2 * H, H], BF16, kind="ExternalInput").ap()
    b1 = nc.dram_tensor("b1", [H, 1], F32, kind="ExternalInput").ap()
    w2 = nc.dram_tensor("w2", [H, 1], BF16, kind="ExternalInput").ap()
    b2 = nc.dram_tensor("b2", [1, 1], F32, kind="ExternalInput").ap()
    out = nc.dram_tensor("out", [1, e_pad], F32, kind="ExternalOutput").ap()

    relu = mybir.ActivationFunctionType.Relu
    ident = mybir.ActivationFunctionType.Identity
    add_op = mybir.AluOpType.add
    max_op = mybir.AluOpType.max

    with tile.TileContext(nc) as tc:
        with (
            tc.tile_pool(name="const", bufs=1) as cpool,
            tc.tile_pool(name="idx", bufs=1) as ipool,
            tc.tile_pool(name="gather", bufs=2) as gpool,
            tc.tile_pool(name="h", bufs=2) as hpool,
            tc.tile_pool(name="stage", bufs=2) as spool,
            tc.tile_pool(name="fc1ps", bufs=4, space="PSUM") as fc1pool,
            tc.tile_pool(name="fc2ps", bufs=3, space="PSUM") as fc2pool,
        ):
            # ---- constants ----
            w1nc = cpool.tile([H, H], BF16, tag="w1nc")
            nc.sync.dma_start(w1nc[:], w1[0:H, :])
            w1pr = cpool.tile([H, H], BF16, tag="w1pr")
            nc.sync.dma_start(w1pr[:], w1[H : 2 * H, :])
            b1_sb = cpool.tile([H, 1], F32, tag="b1")
            nc.sync.dma_start(b1_sb[:], b1[:])
            w2_sb = cpool.tile([H, 1], BF16, tag="w2")
            nc.sync.dma_start(w2_sb[:], w2[:])
            b2_sb = cpool.tile([1, 1], F32, tag="b2")
            nc.sync.dma_start(b2_sb[:], b2[:])

            # ---- indices: replicate [16, N] across the 8 partition groups ----
            idx0_sb = ipool.tile([128, e_pad // 16], I16, tag="idx0")
            idx1_sb = ipool.tile([128, e_pad // 16], I16, tag="idx1")
            for k in range(8):
                nc.sync.dma_start(idx0_sb[16 * k : 16 * (k + 1), :], idx0[:])
                nc.sync.dma_start(idx1_sb[16 * k : 16 * (k + 1), :], idx1[:])

            ic = t_gather // 16  # idx columns per gather tile

            for t in [t for _ in range(reps) for t in range(n_tiles)]:
                g_nc = gpool.tile([H, t_gather], BF16, tag="g_nc")
                nc.gpsimd.dma_gather(
                    g_nc[:].rearrange("p (one t) -> p one t", one=1),
                    t_nc,
                    idx0_sb[:, t * ic : (t + 1) * ic],
                    t_gather,
                    t_gather,
                    H,
                    transpose=True,
                    single_packet=(t_gather <= 512),
                )
                g_pr = gpool.tile([H, t_gather], BF16, tag="g_pr")
                nc.gpsimd.dma_gather(
                    g_pr[:].rearrange("p (one t) -> p one t", one=1),
                    t_pr,
                    idx1_sb[:, t * ic : (t + 1) * ic],
                    t_gather,
                    t_gather,
                    H,
                    transpose=True,
                    single_packet=(t_gather <= 512),
                )

                h_sb = hpool.tile([H, t_gather], BF16, tag="h")
                stage = spool.tile([1, t_gather], F32, tag="stage")

                for c in range(t_gather // chunk):
                    sl = slice(c * chunk, (c + 1) * chunk)
                    ps = fc1pool.tile([H, chunk], F32, tag="fc1")
                    nc.tensor.matmul(
                        ps[:], w1nc[:], g_nc[:, sl], start=True, stop=False
                    )
                    nc.tensor.matmul(
                        ps[:], w1pr[:], g_pr[:, sl], start=False, stop=True
                    )
                    # relu(ps + b1) -> bf16, alternating engines
                    if c % 2 == 0:
                        nc.scalar.activation(h_sb[:, sl], ps[:], relu, bias=b1_sb[:])
                    else:
                        nc.vector.tensor_scalar(
                            h_sb[:, sl], ps[:], b1_sb[:], 0.0, add_op, max_op
                        )

                    ps2 = fc2pool.tile([1, chunk], F32, tag="fc2")
                    nc.tensor.matmul(
                        ps2[:], w2_sb[:], h_sb[:, sl], start=True, stop=True
                    )
                    # stage = ps2 + b2, opposite-parity engines
                    if c % 2 == 0:
                        nc.vector.tensor_scalar(
                            stage[:, sl], ps2[:], b2_sb[:], None, add_op
                        )
                    else:
                        nc.scalar.activation(stage[:, sl], ps2[:], ident, bias=b2_sb[:])

                nc.sync.dma_start(out[:, t * t_gather : (t + 1) * t_gather], stage[:])

    nc.compile()
    return nc


# ---------------------------------------------------------------------------
# Host-side wrapper
# ---------------------------------------------------------------------------

_CACHE: dict = {}


def _wrap_idx(idx: np.ndarray, e_pad: int) -> np.ndarray:
    """int16 [16, e_pad//16] with index i at [i % 16, i // 16]."""
    pad = np.zeros(e_pad, np.int16)
    pad[: idx.shape[0]] = idx.astype(np.int16)
    return np.ascontiguousarray(pad.reshape(e_pad // 16, 16).T)


def _get_program(n_nodes, e_pad, t_gather, chunk):
    key = (n_nodes, e_pad, t_gather, chunk)
    if key not in _CACHE:
        _CACHE[key] = _build(n_nodes, e_pad, t_gather, chunk)
    return _CACHE[key]


def kernel(
    x_ncRNA: np.ndarray,
    x_Protein: np.ndarray,
    edge_label_index: np.ndarray,
    W1: np.ndarray,
    b1: np.ndarray,
    W2: np.ndarray,
    b2: np.ndarray,
    _t_gather: int = 8192,
    _chunk: int = 512,
    _trace: bool = False,
) -> np.ndarray:
    E = edge_label_index.shape[1]
    n_nodes = x_ncRNA.shape[0]
    assert E % N_CORES == 0
    e_core = E // N_CORES
    n_tiles = math.ceil(e_core / _t_gather)
    e_pad = n_tiles * _t_gather

    nc = _get_program(n_nodes, e_pad, _t_gather, _chunk)

    t_nc = np.ascontiguousarray(x_ncRNA.astype(ml_dtypes.bfloat16))
    t_pr = np.ascontiguousarray(x_Protein.astype(ml_dtypes.bfloat16))
    w1 = np.ascontiguousarray(W1.astype(ml_dtypes.bfloat16))
    w2 = np.ascontiguousarray(W2.astype(ml_dtypes.bfloat16))
    b1_ = np.ascontiguousarray(b1.reshape(H, 1).astype(np.float32))
    b2_ = np.ascontiguousarray(b2.reshape(1, 1).astype(np.float32))

    in_maps = []
    for c in range(N_CORES):
        sl = slice(c * e_core, (c + 1) * e_core)
        in_maps.append(
            {
                "t_nc": t_nc,
                "t_pr": t_pr,
                "idx0": _wrap_idx(np.asarray(edge_label_index[0, sl]), e_pad),
                "idx1": _wrap_idx(np.asarray(edge_label_index[1, sl]), e_pad),
                "w1": w1,
                "b1": b1_,
                "w2": w2,
                "b2": b2_,
            }
        )

    res = bass_utils.run_bass_kernel_spmd(
        nc, in_maps, core_ids=list(range(N_CORES)), trace=_trace
    )
    out = np.empty(E, np.float32)
    for c in range(N_CORES):
        out[c * e_core : (c + 1) * e_core] = res.results[c]["out"][0, :e_core]
    kernel._last_results = res
    return out

